# revision 1
# baseline (speedup 1.0000x reference)
"""Trainium2 Bass kernel for nn_DynamicUpsamplePAC.

Math (per batch item, fully data-parallel over B=8 -> 8 NeuronCores):
  1. x_d = bilinear_downsample(x, 160->128 both dims)  (torch align_corners=False)
  2. kern[ij,h,w] = exp(-0.5 * ||g[:,h+i-2,w+j-2] - g[:,h,w]||^2)  (zero-padded guide)
  3. out[o,h,w] = bias[o] + sum_ij kern[ij,h,w] * sum_c x_up[c,...] W[c,o,i,j]
     where x_up is the stride-2 zero-upsampled x_d.  Exploiting the zeros,
     output pixels split into 4 parity classes (h%2,w%2); each class sees only
     taps with matching parity (9/6/6/4 of the 25), reading x_d directly at
     row/col shifts dt,ds in {-1,0,1}.

Kernel layout (per core):
  - x_d held as a zero-padded (128c, 130x130) fp16 SBUF tile.
  - Tap responses live in a 30-slot PSUM grid indexed (alpha-region, g=dt-idx,
    h=ds-idx, beta): slot = region + g*6 + h*2 + beta.  Both the per-shift
    matmul writes (beta-contiguous pairs, single bank) and the per-class
    gathers (uniform g/h strides) are affine in this grid.
  - Per input column w' (128): 15 matmuls (stationary = shifted column view of
    x_d, moving = beta-packed fp16 weights) fill the grid; ScalarE copies each
    class's slots PSUM->SBUF fp16; VectorE multiplies in-place by the
    broadcast pac-kernel values and tensor_reduces over taps into per-parity
    fp32 output tiles; bias is added at the end on GpSimd.
  - Output DMA'd with h-parity interleave, w-contiguous 1KB runs.
"""

import numpy as np

B, C_IN, C_OUT, K = 8, 128, 64, 5
H_IN = W_IN = 160
HD = WD = 128
T = 256
N_CORES = 8

# bilinear 160->128: output row p = 4q+r uses input rows 5q+r, 5q+r+1 with frac f_r
_FRACS = [0.125, 0.375, 0.625, 0.875]

CLASSES = [(0, 0), (0, 1), (1, 0), (1, 1)]  # (h%2, w%2)


def grid_slot(al, be, dt, ds):
    """Slot index in the 30-slot (alpha-region, g, h, beta) grid."""
    g = dt + 1 if al == 0 else dt  # al==1 has dt in {0,1}
    h = ds + 1
    return (0 if al == 0 else 18) + g * 6 + h * 2 + be


def _make_slots():
    """All 25 active (class, tap) combos with shifts + guide-plane mapping."""
    slots = []
    for ci, (al, be) in enumerate(CLASSES):
        for i in range(5):
            if i % 2 != al:
                continue
            for j in range(5):
                if j % 2 != be:
                    continue
                dt = (al + i - 2) // 2  # x_d row shift
                ds = (be + j - 2) // 2  # x_d col shift
                al2 = (al + i) % 2
                be2 = (be + j) % 2
                gda = (al + i - 2 - al2) // 2
                gdw = (be + j - 2 - be2) // 2
                slots.append(
                    dict(ci=ci, al=al, be=be, i=i, j=j, dt=dt, ds=ds,
                         s=grid_slot(al, be, dt, ds),
                         gpar=(al2, be2), gda=gda, gdw=gdw)
                )
    assert len(slots) == 25
    assert len({sl["s"] for sl in slots}) == 25
    return slots


SLOTS = _make_slots()

# per-class gather info: (al, be, region_base, n_g, h0, n_h, prod_base)
CLS_INFO = []
_pbase = 0
for _al, _be in CLASSES:
    _ng = 3 if _al == 0 else 2
    _h0 = 0 if _be == 0 else 1
    _nh = 3 - _h0
    _m = _ng * _nh
    _mp = _m + (_m % 2)
    CLS_INFO.append(dict(al=_al, be=_be, base=0 if _al == 0 else 18,
                         ng=_ng, h0=_h0, nh=_nh, m=_m, mp=_mp, pbase=_pbase))
    _pbase += _mp * 64
assert _pbase == 1664


def _build_program(reps=1):
    import concourse.bass as bass  # noqa: F401
    import concourse.mybir as mybir
    import concourse.tile as tile
    from concourse import bacc

    f32 = mybir.dt.float32
    f16 = mybir.dt.float16
    Alu = mybir.AluOpType
    Act = mybir.ActivationFunctionType

    nc = bacc.Bacc("TRN2", target_bir_lowering=False, debug=False,
                   enable_asserts=False)

    x_d = nc.dram_tensor("x", [C_IN, H_IN, W_IN], f32, kind="ExternalInput").ap()
    g_d = nc.dram_tensor("guide", [3, T, T], f32, kind="ExternalInput").ap()
    w_d = nc.dram_tensor("weight", [C_IN, C_OUT, K, K], f32, kind="ExternalInput").ap()
    b_d = nc.dram_tensor("bias", [C_OUT], f32, kind="ExternalInput").ap()
    o_d = nc.dram_tensor("out", [C_OUT, T, T], f32, kind="ExternalOutput").ap()

    with tile.TileContext(nc) as tc:
      for _rep in range(reps):
        with tc.tile_pool(name="pers", bufs=1) as pers:
            xd = pers.tile([128, 130 * 130], f16, tag="xd")
            kern = pers.tile([128, 30 * 128], f16, tag="kern")
            wb = pers.tile([128, 30 * 64], f16, tag="wb")
            bias_bc = pers.tile([128, 64], f32, tag="bias")
            xd3 = xd[:].rearrange("p (t s) -> p t s", s=130)
            kern3 = kern[:].rearrange("p (sl w) -> p sl w", w=128)
            wb3 = wb[:].rearrange("p (sl o) -> p sl o", o=64)

            # ---- weights (contig load + ACT reorder/cast), bias broadcast ----
            wst = pers.tile([128, 1600], f32, tag="wst")
            nc.sync.dma_start(out=wst[:],
                              in_=w_d.rearrange("c o i j -> c (o i j)"))
            wst4 = wst[:].rearrange("p (o i j) -> p o i j", i=K, j=K)
            for sl in SLOTS:
                nc.scalar.copy(out=wb3[:, sl["s"], :],
                               in_=wst4[:, :, sl["i"], sl["j"]])
            b_bcast_src = b_d.rearrange("(o a) -> o a", a=1) \
                             .broadcast_to([64, 128]).rearrange("o r -> r o")
            nc.sync.dma_start(out=bias_bc[:], in_=b_bcast_src)

            # ---- phase A: bilinear downsample -> xd (fp16, 1px zero border) ----
            nc.gpsimd.memset(xd[:], 0.0)
            with tc.tile_pool(name="phA", bufs=2) as pA:
                for hq in range(4):
                    xh = pA.tile([128, 40 * 160], f32, tag="xh")
                    xh3 = xh[:].rearrange("p (r w) -> p r w", w=160)
                    nc.sync.dma_start(out=xh3, in_=x_d[:, 40 * hq:40 * hq + 40, :])
                    th = pA.tile([128, 32 * 160], f32, tag="th")
                    th3 = th[:].rearrange("p (q r w) -> p q r w", r=4, w=160)
                    for r in range(4):
                        c_r = (1.0 - _FRACS[r]) / _FRACS[r]
                        a_v = xh3[:, r:r + 36:5, :]
                        b_v = xh3[:, r + 1:r + 37:5, :]
                        nc.vector.scalar_tensor_tensor(
                            out=th3[:, :, r, :], in0=a_v, scalar=c_r, in1=b_v,
                            op0=Alu.mult, op1=Alu.add)
                    for r in range(4):
                        for rw in range(4):
                            g_rw = _FRACS[rw]
                            c2 = (1.0 - g_rw) / g_rw
                            aw = th3[:, :, r, rw:rw + 156:5]
                            bw = th3[:, :, r, rw + 1:rw + 157:5]
                            tmp = pA.tile([128, 8 * 32], f32, tag="tmpw")
                            tmp3 = tmp[:].rearrange("p (q m) -> p q m", m=32)
                            nc.vector.scalar_tensor_tensor(
                                out=tmp3, in0=aw, scalar=c2, in1=bw,
                                op0=Alu.mult, op1=Alu.add)
                            base = 32 * hq + r + 1
                            dst = xd3[:, base:base + 30:4, rw + 1:rw + 126:4]
                            nc.scalar.activation(out=dst, in_=tmp3, func=Act.Copy,
                                                 scale=float(_FRACS[r] * g_rw))

            # ---- phase B: pac kernel -> kern (30-slot grid x 128 w', fp16) ----
            # guide loaded with ONE contiguous DMA (1KB runs); parity planes
            # extracted by strided engine copies (DMA would 4-byte-gather).
            with tc.tile_pool(name="phB", bufs=1) as pB:
                gt = pB.tile([128, 3 * 2 * 256], f32, tag="gt")
                gt4 = gt[:].rearrange("p (c al w) -> p c al w", c=3, al=2)
                nc.sync.dma_start(
                    out=gt4, in_=g_d.rearrange("c (a al) w -> a c al w", al=2))
                planes = {}
                idx = 0
                for par in CLASSES:
                    for c in range(3):
                        pl0 = pB.tile([128, 130], f32,
                                      tag=f"pl{par[0]}{par[1]}_{c}_0",
                                      name="pl0")
                        nc.gpsimd.memset(pl0[:], 0.0)
                        src = gt4[:, c, par[0], par[1]:par[1] + 255:2]
                        if idx % 2:
                            nc.gpsimd.tensor_copy(pl0[:, 1:129], src)
                        else:
                            nc.scalar.copy(out=pl0[:, 1:129], in_=src)
                        idx += 1
                        planes[(par, c, 0)] = pl0
                        for da in (-1, 1):
                            pl = pB.tile([128, 130], f32,
                                         tag=f"pl{par[0]}{par[1]}_{c}_{da}",
                                         name="plshift")
                            p0, p1 = max(0, -da), 128 - max(0, da)
                            nc.gpsimd.memset(pl[:], 0.0)
                            nc.sync.dma_start(out=pl[p0:p1, :],
                                              in_=pl0[p0 + da:p1 + da, :])
                            planes[(par, c, da)] = pl
                n2s = {}
                with tc.tile_pool(name="phB2", bufs=3) as pB2:
                    for par in CLASSES:
                        for da in (-1, 0, 1):
                            n2 = pB.tile([128, 130], f32,
                                         tag=f"n2{par[0]}{par[1]}_{da}")
                            tmp = pB2.tile([128, 130], f32, tag="n2tmp")
                            p0 = planes[(par, 0, da)][:]
                            p1_ = planes[(par, 1, da)][:]
                            p2 = planes[(par, 2, da)][:]
                            nc.gpsimd.tensor_mul(n2[:], p0, p0)
                            nc.gpsimd.tensor_mul(tmp[:], p1_, p1_)
                            nc.gpsimd.tensor_add(n2[:], n2[:], tmp[:])
                            nc.gpsimd.tensor_mul(tmp[:], p2, p2)
                            nc.gpsimd.tensor_add(n2[:], n2[:], tmp[:])
                            n2s[(par, da)] = n2
                    for sl in SLOTS:
                        par, gpar = (sl["al"], sl["be"]), sl["gpar"]
                        gda, gdw = sl["gda"], sl["gdw"]
                        dot = pB2.tile([128, 128], f32, tag="dot")
                        tmp = pB2.tile([128, 128], f32, tag="dtmp")
                        for c in range(3):
                            nb = planes[(gpar, c, gda)][:, 1 + gdw:129 + gdw]
                            ce = planes[(par, c, 0)][:, 1:129]
                            if c == 0:
                                nc.gpsimd.tensor_mul(dot[:], nb, ce)
                            else:
                                nc.gpsimd.tensor_mul(tmp[:], nb, ce)
                                nc.gpsimd.tensor_add(dot[:], dot[:], tmp[:])
                        # t = 0.5*n2_nb - dot ; t = 0.5*n2_c + t ; kern = exp(-t)
                        nc.vector.scalar_tensor_tensor(
                            out=dot[:], in0=n2s[(gpar, gda)][:, 1 + gdw:129 + gdw],
                            scalar=0.5, in1=dot[:], op0=Alu.mult, op1=Alu.subtract)
                        nc.vector.scalar_tensor_tensor(
                            out=dot[:], in0=n2s[(par, 0)][:, 1:129],
                            scalar=0.5, in1=dot[:], op0=Alu.mult, op1=Alu.add)
                        nc.scalar.activation(out=kern3[:, sl["s"], :], in_=dot[:],
                                             func=Act.Exp, scale=-1.0)

            # ---- phase C: per-column matmuls + kern apply + tap reduce ----
            # acc: fp16, w-major (a, [w 256, o 64]) so reduce outputs are
            # contiguous fp16 (2x DVE mode); converted to fp32 on ScalarE at
            # the end in w-chunks and DMA'd out.
            with tc.tile_pool(name="acc", bufs=1) as pacc, \
                 tc.tile_pool(name="prod", bufs=3) as pprod, \
                 tc.tile_pool(name="psum", bufs=2, space="PSUM") as pps:
                accs = [pacc.tile([128, 256 * 64], f16, tag=f"acc{al}",
                                  name=f"acc{al}")
                        for al in range(2)]
                acc4 = [a.rearrange("p (w o) -> p w o", o=64) for a in accs]
                for wp in range(128):
                    ps = pps.tile([128, 30 * 64], f32, tag="ps")
                    ps3 = ps.rearrange("p (sl o) -> p sl o", o=64)
                    for dt in (-1, 0, 1):
                        for ds in (-1, 0, 1):
                            lhsT = xd3[:, 1 + dt:129 + dt, 1 + wp + ds]
                            nb = 1 if ds == -1 else 2
                            for al in range(2):
                                if al == 1 and dt == -1:
                                    continue
                                s0 = grid_slot(al, 0, dt, ds)
                                nc.tensor.matmul(
                                    out=ps3[:, s0:s0 + nb, :],
                                    lhsT=lhsT,
                                    rhs=wb3[:, s0:s0 + nb, :],
                                    start=True, stop=True)
                    prod = pprod.tile([128, 1664], f16, tag="prod")
                    kx = pprod.tile([128, 1664], f16, tag="kx")
                    # zero the ee pad lane in both (keeps mult/reduce finite)
                    nc.gpsimd.memset(
                        prod[:, 0:640].rearrange("p (o t) -> p o t", t=10)[:, :, 9],
                        0.0)
                    nc.gpsimd.memset(
                        kx[:, 0:640].rearrange("p (o t) -> p o t", t=10)[:, :, 9],
                        0.0)
                    for ci in CLS_INFO:
                        ng, h0, nh = ci["ng"], ci["h0"], ci["nh"]
                        m, mp = ci["m"], ci["mp"]
                        reg = ci["base"]
                        srcv = ps[:, reg * 64:(reg + ng * 6) * 64].rearrange(
                            "p (g h b o) -> p g h b o", g=ng, h=3, b=2, o=64
                        )[:, :, h0:, ci["be"], :]
                        pr = prod[:, ci["pbase"]:ci["pbase"] + mp * 64]
                        dstv = pr.rearrange(
                            "p (o gh) -> p o gh", gh=mp)[:, :, :m].rearrange(
                            "p o (g h) -> p g h o", g=ng, h=nh)
                        nc.scalar.copy(out=dstv, in_=srcv)
                        # kern values expanded over o on gpsimd (same layout)
                        kv = kern[:, reg * 128:(reg + ng * 6) * 128].rearrange(
                            "p (g h b w) -> p g h b w", g=ng, h=3, b=2, w=128
                        )[:, :, h0:, ci["be"], wp:wp + 1].broadcast_to(
                            [128, ng, nh, 64])
                        kxv = kx[:, ci["pbase"]:ci["pbase"] + mp * 64].rearrange(
                            "p (o gh) -> p o gh", gh=mp)[:, :, :m].rearrange(
                            "p o (g h) -> p g h o", g=ng, h=nh)
                        nc.gpsimd.tensor_copy(kxv, kv)
                    nc.vector.tensor_mul(prod[:], prod[:], kx[:])
                    with nc.allow_low_precision("fp16 tap-sum, fp32 dyn range ok"):
                        for ci in CLS_INFO:
                            rin = prod[:, ci["pbase"]:ci["pbase"] + ci["mp"] * 64] \
                                .rearrange("p (o t) -> p o t", t=ci["mp"])
                            nc.vector.tensor_reduce(
                                out=acc4[ci["al"]][:, 2 * wp + ci["be"], :],
                                in_=rin, axis=mybir.AxisListType.X, op=Alu.add)
                # bias add (gpsimd, fp16 acc, step-0 broadcast along w)
                bias16 = pacc.tile([128, 64], f16, tag="bias16")
                nc.gpsimd.tensor_copy(bias16[:], bias_bc[:])
                for al in range(2):
                    bv = bias16.rearrange("p (w o) -> p w o", w=1) \
                               .broadcast_to([128, 256, 64])
                    nc.gpsimd.tensor_add(acc4[al], acc4[al], bv)
                # fp16 -> fp32 convert (permute to o-major) + DMA out, w-chunks
                with tc.tile_pool(name="stg", bufs=2) as pstg:
                    for al in range(2):
                        for half in range(2):
                            stg = pstg.tile([128, 64 * 128], f32, tag="stg")
                            stg3 = stg.rearrange("p (o w) -> p o w", w=128)
                            src_v = acc4[al][:, half * 128:(half + 1) * 128, :]
                            nc.scalar.copy(
                                out=stg3.rearrange("p o w -> p w o"), in_=src_v)
                            dst = o_d.rearrange(
                                "o (a p) (wh w) -> p a wh o w", p=2, wh=2
                            )[al][:, half]
                            nc.scalar.dma_start(out=dst, in_=stg3)
    nc.compile()
    return nc


_CACHE = {}


def _get_nc():
    if "nc" not in _CACHE:
        _CACHE["nc"] = _build_program()
    return _CACHE["nc"]


def kernel(x, guide, weight, bias, target_size=None, **_unused):
    from concourse.bass_utils import run_bass_kernel_spmd

    nc = _get_nc()
    x = np.ascontiguousarray(np.asarray(x, dtype=np.float32))
    guide = np.ascontiguousarray(np.asarray(guide, dtype=np.float32))
    weight = np.ascontiguousarray(np.asarray(weight, dtype=np.float32))
    bias = np.ascontiguousarray(np.asarray(bias, dtype=np.float32))
    in_maps = [
        {"x": x[b], "guide": guide[b], "weight": weight, "bias": bias}
        for b in range(B)
    ]
    res = run_bass_kernel_spmd(nc, in_maps, list(range(N_CORES))).results
    return np.stack([res[b]["out"] for b in range(B)], axis=0).astype(np.float32)

